# revision 1
# baseline (speedup 1.0000x reference)
"""Multi-head self-attention (B=4, S=2048, D=512, H=8, d=64) on 8 trn2 cores.

Sharding: 2 cores per batch element; each core computes 4 heads (a 256-wide
column slice of Wq/Wk/Wv and row slice of Wh) and produces a partial
[S, 512] output; the host sums the two partials per batch and adds bh.

Per-core pipeline (matmul operands float32r — fp32 data streamed at bf16
rate with ~13-bit-mantissa input rounding; x^T is pre-transposed on the
host and DMAed straight into an f32r tile, so nothing is staged or cast):
  A) project qT/kT [256, S] and v [S, 256] from the resident x^T
     (v stored augmented with a ones column per head so the attention
     matmul also produces the softmax denominator row).
  B) heads processed in pairs packed on PE row-groups (K=64 each at base
     partitions 0/64): scores^T -> one FD-1024 exp on ScalarE covering
     both heads (no max-subtraction — logits are O(5)); the augmented-V
     matmul accumulates attnU^T[d,sq] + the Z row; normalize via DVE
     reciprocal + K=1 ones-matmul broadcast + tensor_tensor multiply.
  C) out[s,:] = attnT.T @ Wh accumulated over the 256 head dims,
     emitted one quarter behind attention so PE's in-order queue never
     delays the next quarter's scores.

ScalarE's exp stream (~133us busy at ~97% density) is the bottleneck and
shadows all matmul/DMA/normalization work; measured ~245us end-to-end.
"""

import numpy as np

NUM_HEADS = 8
D_MODEL = 512
D_HEAD = 64
B = 4
S = 2048
H_PER_CORE = 4          # heads per core
DQ = H_PER_CORE * D_HEAD  # 256 = per-core q/k/v width
N_CORES = 8
SCALE = 1.0 / np.sqrt(D_HEAD)

_KO = D_MODEL // 128    # 4 contraction chunks for the projections
_NT = S // 128          # 16 tiles of 128 along S
_VW = D_HEAD + 1        # 65: v columns per head incl. ones column


def _split_excess_waits(nc):
    """Walrus's TRN2 codegen fits very few sync-waits per instruction (one on
    a Matmult's weight-load, few on drains).  Move excess waits onto NoOps
    inserted just before the instruction — engine queues are in-order, so a
    wait on a preceding same-engine instruction still protects it."""
    import concourse.mybir as mybir

    n_fixed = 0
    for f in nc.m.functions:
        for bb in f.blocks:
            insts = list(bb.instructions)
            out = []
            changed = False
            for ins in insts:
                si = ins.sync_info
                if si is not None and si.on_wait and len(si.on_wait) > 1:
                    waits = list(si.on_wait)
                    # An exp/matmul waiting on its OWN engine's completion sem
                    # is a slot-recycle WAW guard: implied by in-order issue,
                    # with the interleaved cross-engine reader guarded by the
                    # remaining wait.  Dropping it avoids a NoOp on the
                    # bottleneck queue (one per exp otherwise).
                    if isinstance(ins, (mybir.InstActivation, mybir.InstMatmult)):
                        eng_pfx = str(ins.engine).split(".")[-1] + "_"
                        cross = [w for w in waits
                                 if not str(getattr(w, "ant_name", "")).startswith(eng_pfx)]
                        if cross and len(cross) < len(waits):
                            waits = cross
                    for j, w in enumerate(waits[1:]):
                        nop = mybir.InstNoOp(
                            name=f"{ins.name}_waitnop{j}", ins=[], outs=[])
                        nop.engine = ins.engine
                        nop.sync_info = mybir.SyncInfo(on_wait=[w], on_update=[])
                        out.append(nop)
                    ins.sync_info = mybir.SyncInfo(
                        on_wait=waits[:1], on_update=list(si.on_update or []))
                    n_fixed += 1
                    changed = True
                out.append(ins)
            if changed:
                bb.instructions = out
    return n_fixed


def build_nc(nrep=1):
    """Build the per-core Bass program.  nrep>1 repeats the compute body
    (same tiles, idempotent) for wall-clock timing amplification."""
    import concourse.bass as bass
    import concourse.mybir as mybir
    import concourse.tile as tile
    from concourse.masks import make_identity

    f32 = mybir.dt.float32
    f32r = mybir.dt.float32r
    AF = mybir.ActivationFunctionType

    nc = bass.Bass()
    x_d = nc.dram_tensor("x", [D_MODEL, S], f32r, kind="ExternalInput")
    wq_d = nc.dram_tensor("wq", [D_MODEL, DQ], f32, kind="ExternalInput")
    wk_d = nc.dram_tensor("wk", [D_MODEL, DQ], f32, kind="ExternalInput")
    wv_d = nc.dram_tensor("wv", [D_MODEL, DQ], f32, kind="ExternalInput")
    wh_d = nc.dram_tensor("wh", [DQ, D_MODEL], f32, kind="ExternalInput")
    bq_d = nc.dram_tensor("bq", [DQ], f32, kind="ExternalInput")
    bk_d = nc.dram_tensor("bk", [DQ], f32, kind="ExternalInput")
    bv_d = nc.dram_tensor("bv", [DQ], f32, kind="ExternalInput")
    o_d = nc.dram_tensor("o", [S, D_MODEL], f32, kind="ExternalOutput")

    with (
        nc.allow_low_precision(reason="float32r attention pipeline"),
        tile.TileContext(nc) as tc,
        tc.tile_pool(name="cst", bufs=1) as cst,
        tc.tile_pool(name="big", bufs=1) as big,
        tc.tile_pool(name="pr", bufs=4) as pr,
        tc.tile_pool(name="ps_sc", bufs=2, space="PSUM") as ps_sc,
        tc.tile_pool(name="ps_av", bufs=3, space="PSUM") as ps_av,
        tc.tile_pool(name="ps_bc", bufs=1, space="PSUM") as ps_bc,
    ):
        ones_col = cst.tile([1, D_HEAD], f32r)
        nc.gpsimd.memset(ones_col.bitcast(f32)[:], 1.0)

        # ---- load + cast weights/biases (once) ----
        w_sb = {}
        for name, dram, shp in (
            ("wq", wq_d, (128, _KO, DQ)),
            ("wk", wk_d, (128, _KO, DQ)),
            ("wv", wv_d, (128, _KO, DQ)),
            ("wh", wh_d, (128, DQ // 128, D_MODEL)),
        ):
            raw = pr.tile(list(shp), f32, tag="wraw")
            nc.scalar.dma_start(raw[:], dram.rearrange("(a p) m -> p a m", p=128))
            cvt = big.tile(list(shp), f32r, tag=f"w_{name}")
            nc.vector.tensor_copy(cvt[:], raw[:])
            w_sb[name] = cvt
        bias_sb = {}
        for name, dram in (("bq", bq_d), ("bk", bk_d), ("bv", bv_d)):
            bt = cst.tile([128, DQ // 128], f32, tag=f"b_{name}")
            nc.scalar.dma_start(bt[:], dram.rearrange("(o p) -> p o", p=128))
            bias_sb[name] = bt

        # ---- load x^T (host pre-transposed) straight into f32r ----
        xT0 = big.tile([128, _KO, S], f32r, tag="xT")
        x_engs = [nc.sync, nc.gpsimd, nc.scalar, nc.sync,
                  nc.gpsimd, nc.scalar, nc.sync, nc.gpsimd]
        for sg in range(8):
            x_engs[sg].dma_start(
                xT0[:, :, sg * 256:(sg + 1) * 256],
                x_d.rearrange("(a p) s -> p a s", p=128)[:, :, sg * 256:(sg + 1) * 256])

        for _rep in range(nrep):
            xT = xT0
            qT = big.tile([128, DQ // 128, S], f32r, tag="qT")
            kT = big.tile([128, DQ // 128, S], f32r, tag="kT")
            attnT = big.tile([128, DQ // 128, S], f32r, tag="attnT")
            v_aug = big.tile([128, _NT, H_PER_CORE * _VW], f32r, tag="v_aug")
            nc.gpsimd.memset(v_aug.bitcast(f32)[:], 1.0)

            def transpose_group(tg):
                pass  # x^T already resident in f32r

            def proj_group(dst, wname, bname, o, sg, on_act=False):
                p = ps_sc.tile([128, 1024], f32, tag="sc", name="p_qk")
                for ko in range(_KO):
                    nc.tensor.matmul(
                        p[:, :512],
                        w_sb[wname][:, ko, o * 128:(o + 1) * 128],
                        xT[:, ko, sg * 512:(sg + 1) * 512],
                        start=(ko == 0), stop=(ko == _KO - 1))
                if on_act:
                    nc.scalar.activation(
                        dst[:, o, sg * 512:(sg + 1) * 512], p[:, :512],
                        AF.Identity, bias=bias_sb[bname][:, o:o + 1])
                else:
                    nc.vector.tensor_scalar_add(
                        dst[:, o, sg * 512:(sg + 1) * 512], p[:, :512],
                        bias_sb[bname][:, o:o + 1])

            def project_v(t0, t1):
                for t in range(t0, t1):
                    pv = ps_av.tile([128, 512], f32, tag="av", name="pv")
                    for ko in range(_KO):
                        nc.tensor.matmul(
                            pv[:, :DQ],
                            xT[:, ko, t * 128:(t + 1) * 128],
                            w_sb["wv"][:, ko, :],
                            start=(ko == 0), stop=(ko == _KO - 1))
                    nc.vector.tensor_copy(
                        v_aug[:, t, :].rearrange(
                            "p (h w) -> p h w", w=_VW)[:, :, :D_HEAD],
                        pv[:, :DQ].rearrange("p (h w) -> p h w", w=D_HEAD))

            def attend_pair_quarter(o, jq, t0=0, t1=_NT, avs=None):
                # heads (2o, 2o+1) at base partitions 0 / 64, packed on PE
                # row groups; each sc tile is [t=128, h0-sq512 | h1-sq512].
                h0, h1 = 2 * o, 2 * o + 1
                sq = jq * 512
                if avs is None:
                    av0 = ps_av.tile([128, 512], f32, tag="av", name="av0")
                    av1 = ps_av.tile([128, 512], f32, tag="av", name="av1")
                else:
                    av0, av1 = avs
                # Software-pipelined by one t-step: av(t) needs exp(t), so
                # emit sc(t+1) ahead of av(t) — PE computes the next tile's
                # scores while ScalarE exps this one, instead of blocking
                # in-order on the exp result.
                def emit_av(t, probs):
                    nc.tensor.matmul(
                        av0[0:_VW, :],
                        v_aug[:, t, h0 * _VW:(h0 + 1) * _VW],
                        probs[:, 0:512],
                        start=(t == 0), stop=(t == _NT - 1))
                    nc.tensor.matmul(
                        av1[0:_VW, :],
                        v_aug[:, t, h1 * _VW:(h1 + 1) * _VW],
                        probs[:, 512:1024],
                        start=(t == 0), stop=(t == _NT - 1))

                pending = None
                for t in range(t0, t1):
                    sc = ps_sc.tile([128, 1024], f32, tag="sc", name="sc")
                    nc.tensor.matmul(
                        sc[:, 0:512],
                        kT[0:64, o, t * 128:(t + 1) * 128],
                        qT[0:64, o, sq:sq + 512],
                        start=True, stop=True)
                    nc.tensor.matmul(
                        sc[:, 512:1024],
                        kT[64:128, o, t * 128:(t + 1) * 128],
                        qT[64:128, o, sq:sq + 512],
                        start=True, stop=True)
                    probs = pr.tile([128, 1024], f32r, tag="probs")
                    nc.scalar.activation(probs[:], sc[:], AF.Exp,
                                         scale=float(SCALE))
                    if pending is not None:
                        emit_av(*pending)
                    pending = (t, probs)
                emit_av(*pending)
                if t1 < _NT:
                    return (av0, av1)
                for hh, av in ((h0, av0), (h1, av1)):
                    bp = 64 * (hh % 2)
                    invZ = pr.tile([1, 512], f32r, tag="invz")
                    nc.vector.reciprocal(invZ[:], av[D_HEAD:_VW, :])
                    bc = ps_bc.tile([64, 512], f32, tag="bc", name="bc")
                    nc.tensor.matmul(bc[:], ones_col[:], invZ[:],
                                     start=True, stop=True)
                    bc_sb = pr.tile([64, 512], f32, tag="bc_sb")
                    nc.vector.tensor_copy(bc_sb[:], bc[:])
                    nc.vector.tensor_tensor(
                        attnT[bp:bp + 64, o, sq:sq + 512],
                        av[0:D_HEAD, :], bc_sb[:], mybir.AluOpType.mult)

            def out_quarter(jq):
                # out rows [512*jq, 512*jq+512) need attnT for all heads there
                for o in range(DQ // 128):
                    nc.vector.tensor_scalar_add(
                        attnT[:, o, jq * 512:(jq + 1) * 512],
                        attnT[:, o, jq * 512:(jq + 1) * 512],
                        bias_sb["bv"][:, o:o + 1])
                for sg in range(4 * jq, 4 * jq + 4):
                    po = ps_av.tile([128, 512], f32, tag="av", name="po")
                    for o in range(DQ // 128):
                        nc.tensor.matmul(
                            po[:],
                            attnT[:, o, sg * 128:(sg + 1) * 128],
                            w_sb["wh"][:, o, :],
                            start=(o == 0), stop=(o == DQ // 128 - 1))
                    ot = pr.tile([128, 512], f32, tag="ot")
                    nc.vector.tensor_copy(ot[:], po[:])
                    oeng = nc.sync if sg % 2 == 0 else nc.gpsimd
                    oeng.dma_start(
                        o_d.rearrange("(t p) d -> p t d", p=128)[:, sg, :], ot[:])

            transpose_group(0)
            proj_group(kT, "wk", "bk", 0, 0, on_act=True)
            transpose_group(1)
            proj_group(kT, "wk", "bk", 0, 1, on_act=True)
            proj_group(qT, "wq", "bq", 0, 0, on_act=True)
            project_v(0, 4)
            transpose_group(2)
            proj_group(kT, "wk", "bk", 0, 2, on_act=True)
            transpose_group(3)
            proj_group(kT, "wk", "bk", 0, 3, on_act=True)
            project_v(4, 16)
            for sg in (1, 2, 3):
                proj_group(qT, "wq", "bq", 0, sg)
            for sg in range(4):
                proj_group(kT, "wk", "bk", 1, sg)
            for sg in range(4):
                proj_group(qT, "wq", "bq", 1, sg)
            for jq in range(4):
                attend_pair_quarter(0, jq)
                attend_pair_quarter(1, jq)
                if jq > 0:
                    out_quarter(jq - 1)
            out_quarter(3)

    _split_excess_waits(nc)
    return nc


def _in_maps(inputs):
    x = np.ascontiguousarray(np.asarray(inputs["x"], dtype=np.float32))
    maps = []
    for c in range(N_CORES):
        b, g = c // 2, c % 2
        hs = slice(g * DQ, (g + 1) * DQ)
        maps.append({
            "x": np.ascontiguousarray(x[b].T),
            "wq": np.ascontiguousarray(np.asarray(inputs["Wq"], np.float32)[:, hs]),
            "wk": np.ascontiguousarray(np.asarray(inputs["Wk"], np.float32)[:, hs]),
            "wv": np.ascontiguousarray(np.asarray(inputs["Wv"], np.float32)[:, hs]),
            "wh": np.ascontiguousarray(np.asarray(inputs["Wh"], np.float32)[hs, :]),
            "bq": np.ascontiguousarray(np.asarray(inputs["bq"], np.float32)[hs]),
            "bk": np.ascontiguousarray(np.asarray(inputs["bk"], np.float32)[hs]),
            "bv": np.ascontiguousarray(np.asarray(inputs["bv"], np.float32)[hs]),
        })
    return maps


def kernel(**inputs):
    from concourse.bass_utils import run_bass_kernel_spmd

    nc = build_nc(nrep=1)
    maps = _in_maps(inputs)
    res = run_bass_kernel_spmd(nc, maps, core_ids=list(range(N_CORES)))
    bh = np.asarray(inputs["bh"], np.float32)
    out = np.empty((B, S, D_MODEL), np.float32)
    for b in range(B):
        out[b] = res.results[2 * b]["o"] + res.results[2 * b + 1]["o"] + bh
    return out



# revision 13
# speedup vs baseline: 1.0775x; 1.0775x over previous
"""Multi-head self-attention (B=4, S=2048, D=512, H=8, d=64) on 8 trn2 cores.

Sharding: 2 cores per batch element; each core computes 4 heads (a 256-wide
column slice of Wq/Wk/Wv and row slice of Wh) and produces a partial
[S, 512] output; the host sums the two partials per batch and adds bh.

Per-core pipeline (matmul operands float32r — fp32 data streamed at bf16
rate with ~13-bit-mantissa input rounding; x^T is pre-transposed on the
host and DMAed straight into an f32r tile):

ScalarE's exp stream (128 x [128,1024] exps ~ 133us busy) is the hard
floor; the kernel is structured to keep it saturated end-to-end:
  - minimal pre-phase: kT for head-pair 0, qT pair-0 first quarter, all
    of V — just enough to start the first scores tile;
  - every remaining projection (rest of qT, kT pair 1) plus the output
    projection and normalization is decomposed into ~512-row PE closures
    injected between attention steps, budgeted to PE's per-step slack
    behind ScalarE;
  - attention runs pair 0 (all 4 S_q quarters) then pair 1, so pair 1's
    projections have 64 steps of slack to land in;
  - the augmented-V ones column gives the softmax denominator Z as row
    64 of the attention PSUM; both heads' 1/Z rows are staged into a
    [2,512] tile and broadcast with a single K=2 matmul; attnT = av*bc
    on DVE; the av matmul trails exp by 2 steps so quarter-boundary
    normalization never stalls the PSUM bank rotation.
"""

import numpy as np

NUM_HEADS = 8
D_MODEL = 512
D_HEAD = 64
B = 4
S = 2048
H_PER_CORE = 4          # heads per core
DQ = H_PER_CORE * D_HEAD  # 256 = per-core q/k/v width
N_CORES = 8
SCALE = 1.0 / np.sqrt(D_HEAD)

_KO = D_MODEL // 128    # 4 contraction chunks for the projections
_NT = S // 128          # 16 tiles of 128 along S
_VW = D_HEAD + 1        # 65: v columns per head incl. ones column


def _split_excess_waits(nc):
    """Walrus's TRN2 codegen fits very few sync-waits per instruction (one on
    a Matmult's weight-load, few on drains).  Move excess waits onto NoOps
    inserted just before the instruction — engine queues are in-order, so a
    wait on a preceding same-engine instruction still protects it."""
    import concourse.mybir as mybir

    n_fixed = 0
    for f in nc.m.functions:
        for bb in f.blocks:
            insts = list(bb.instructions)
            out = []
            changed = False
            for ins in insts:
                si = ins.sync_info
                if si is not None and si.on_wait and len(si.on_wait) > 1:
                    waits = list(si.on_wait)
                    # An exp/matmul waiting on its OWN engine's completion sem
                    # is a slot-recycle WAW guard: implied by in-order issue,
                    # with the interleaved cross-engine reader guarded by the
                    # remaining wait.  Dropping it avoids a NoOp on the
                    # bottleneck queue (one per exp otherwise).
                    if isinstance(ins, (mybir.InstActivation, mybir.InstMatmult)):
                        eng_pfx = str(ins.engine).split(".")[-1] + "_"
                        cross = [w for w in waits
                                 if not str(getattr(w, "ant_name", "")).startswith(eng_pfx)]
                        if cross and len(cross) < len(waits):
                            waits = cross
                    for j, w in enumerate(waits[1:]):
                        nop = mybir.InstNoOp(
                            name=f"{ins.name}_waitnop{j}", ins=[], outs=[])
                        nop.engine = ins.engine
                        nop.sync_info = mybir.SyncInfo(on_wait=[w], on_update=[])
                        out.append(nop)
                    ins.sync_info = mybir.SyncInfo(
                        on_wait=waits[:1], on_update=list(si.on_update or []))
                    n_fixed += 1
                    changed = True
                out.append(ins)
            if changed:
                bb.instructions = out
    return n_fixed


def build_nc(nrep=1):
    """Build the per-core Bass program.  nrep>1 repeats the compute body
    (same tiles, idempotent) for wall-clock timing amplification."""
    from collections import deque

    import concourse.bass as bass
    import concourse.mybir as mybir
    import concourse.tile as tile

    f32 = mybir.dt.float32
    f32r = mybir.dt.float32r
    bf16 = mybir.dt.bfloat16
    AF = mybir.ActivationFunctionType

    nc = bass.Bass()
    x_d = nc.dram_tensor("x", [D_MODEL, S], bf16, kind="ExternalInput")
    wq_d = nc.dram_tensor("wq", [D_MODEL, DQ], bf16, kind="ExternalInput")
    wk_d = nc.dram_tensor("wk", [D_MODEL, DQ], bf16, kind="ExternalInput")
    wv_d = nc.dram_tensor("wv", [D_MODEL, DQ], bf16, kind="ExternalInput")
    wh_d = nc.dram_tensor("wh", [DQ, D_MODEL], bf16, kind="ExternalInput")
    bq_d = nc.dram_tensor("bq", [DQ], f32, kind="ExternalInput")
    bk_d = nc.dram_tensor("bk", [DQ], f32, kind="ExternalInput")
    bv_d = nc.dram_tensor("bv", [DQ], f32, kind="ExternalInput")
    o_d = nc.dram_tensor("o", [S, D_MODEL], bf16, kind="ExternalOutput")

    with (
        nc.allow_low_precision(reason="float32r attention pipeline"),
        tile.TileContext(nc) as tc,
        tc.tile_pool(name="cst", bufs=1) as cst,
        tc.tile_pool(name="big", bufs=1) as big,
        tc.tile_pool(name="pr", bufs=5) as pr,
        tc.tile_pool(name="ps_sc", bufs=2, space="PSUM") as ps_sc,
        tc.tile_pool(name="ps_av", bufs=3, space="PSUM") as ps_av,
        tc.tile_pool(name="ps_x", bufs=1, space="PSUM") as ps_x,
    ):
        # ones_col [1,64]: K=1 matmul broadcasts a [1,512] row across 64
        # output partitions (denominator broadcast + PE warmup fodder)
        ones_col = cst.tile([1, D_HEAD], f32r)
        nc.gpsimd.memset(ones_col.bitcast(f32)[:], 1.0)

        # ---- DMA schedule: ONE ring (SP) in exact consumption order, so
        # the shared DMA pipe never reorders a late-needed transfer ahead of
        # an early-needed one: wk, x01, bk, wq, x23, bq, wv, x4..x7, bv, wh.
        w_sb = {}
        bias_sb = {}
        xT = big.tile([128, _KO, S], bf16, tag="xT")
        x_r = x_d.rearrange("(a p) s -> p a s", p=128)

        def _load_w(name, dram, shp):
            wt = big.tile(list(shp), bf16, tag=f"w_{name}")
            nc.sync.dma_start(wt[:], dram.rearrange("(a p) m -> p a m", p=128))
            w_sb[name] = wt

        def _load_b(name, dram):
            bt = cst.tile([128, DQ // 128], f32, tag=f"b_{name}")
            nc.sync.dma_start(bt[:], dram.rearrange("(o p) -> p o", p=128))
            bias_sb[name] = bt

        def _load_x(sg):
            nc.sync.dma_start(
                xT[:, :, sg * 256:(sg + 1) * 256],
                x_r[:, :, sg * 256:(sg + 1) * 256])

        _load_w("wk", wk_d, (128, _KO, DQ))
        _load_x(0); _load_x(1)
        _load_b("bk", bk_d)
        _load_w("wq", wq_d, (128, _KO, DQ))
        _load_b("bq", bq_d)
        _load_w("wv", wv_d, (128, _KO, DQ))
        _load_b("bv", bv_d)
        _load_x(2); _load_x(3); _load_x(4); _load_x(5); _load_x(6); _load_x(7)
        _load_w("wh", wh_d, (128, DQ // 128, D_MODEL))

        for _rep in range(nrep):
            qT = big.tile([128, DQ // 128, S], f32r, tag="qT")
            kT = big.tile([128, DQ // 128, S], f32r, tag="kT")
            attnT = big.tile([128, DQ // 128, S], bf16, tag="attnT")
            v_aug = big.tile([128, _NT, H_PER_CORE * _VW], f32r, tag="v_aug")
            # only the per-head ones columns need (re)setting; pv moves
            # overwrite the rest every rep
            nc.gpsimd.memset(
                v_aug.bitcast(f32).rearrange(
                    "p t (h w) -> p t h w", w=_VW)[:, :, :, D_HEAD:], 1.0)

            # ---------- projection helpers ----------
            def proj_group(dst, wname, bname, o, sg, pool):
                """dst[:, o, sg*512:(sg+1)*512] = (W^T x + b), via 4 K-chunks."""
                p = pool.tile([128, 1024], f32, tag="sc", name="p_qk")
                for ko in range(_KO):
                    nc.tensor.matmul(
                        p[:, :512],
                        w_sb[wname][:, ko, o * 128:(o + 1) * 128],
                        xT[:, ko, sg * 512:(sg + 1) * 512],
                        start=(ko == 0), stop=(ko == _KO - 1))
                nc.vector.tensor_scalar_add(
                    dst[:, o, sg * 512:(sg + 1) * 512], p[:, :512],
                    bias_sb[bname][:, o:o + 1])

            def proj_group_closures(dst, wname, bname, o, sg):
                """Same as proj_group but as 4 one-matmul closures (+move on
                the last) against the shared ps_x rotator."""
                box = {}

                def mk(ko):
                    def go():
                        if ko == 0:
                            box["p"] = ps_x.tile([128, 512], f32,
                                                 tag="px", name="p_qk")
                        nc.tensor.matmul(
                            box["p"][:],
                            w_sb[wname][:, ko, o * 128:(o + 1) * 128],
                            xT[:, ko, sg * 512:(sg + 1) * 512],
                            start=(ko == 0), stop=(ko == _KO - 1))
                        if ko == _KO - 1:
                            nc.vector.tensor_scalar_add(
                                dst[:, o, sg * 512:(sg + 1) * 512], box["p"][:],
                                bias_sb[bname][:, o:o + 1])
                    return go
                return [(512, mk(ko)) for ko in range(_KO)]

            def project_v(t, pool):
                pv = pool.tile([128, 1024], f32, tag="sc", name="pv")
                for ko in range(_KO):
                    nc.tensor.matmul(
                        pv[:, :DQ],
                        xT[:, ko, t * 128:(t + 1) * 128],
                        w_sb["wv"][:, ko, :],
                        start=(ko == 0), stop=(ko == _KO - 1))
                nc.vector.tensor_copy(
                    v_aug[:, t, :].rearrange(
                        "p (h w) -> p h w", w=_VW)[:, :, :D_HEAD],
                    pv[:, :DQ].rearrange("p (h w) -> p h w", w=D_HEAD))

            # ---------- filler machinery ----------
            fillers = deque()   # items: (pe_rows, closure)
            spent = [0.0]
            clock = [0.0]       # steps elapsed (1 step = one exp)

            def pump(budget_rows_per_step):
                clock[0] += 1.0
                while fillers and spent[0] < clock[0] * budget_rows_per_step:
                    rows, go = fillers.popleft()
                    go()
                    spent[0] += rows

            def drain_fillers():
                while fillers:
                    rows, go = fillers.popleft()
                    go()
                    spent[0] += rows

            # ---------- attention ----------
            def normalize_closures(o, jq):
                """invZ (DVE) -> K=1 broadcast per head (PE) -> attnT=av*bc
                (DVE).  av0/av1 banks are released by the tt ops."""
                sq = jq * 512
                av0, av1 = cur_av[(o, jq)]
                box = {}

                def go_recip():
                    iz0 = pr.tile([1, 512], f32r, tag="invz")
                    iz1 = pr.tile([1, 512], f32r, tag="invz")
                    nc.vector.reciprocal(iz0[:], av0[D_HEAD:_VW, :])
                    nc.vector.reciprocal(iz1[:], av1[D_HEAD:_VW, :])
                    box["iz"] = (iz0, iz1)

                def go_bc0():
                    # matmul outputs must start at partition 0 here, so each
                    # head's K=1 broadcast lands at rows 0:64 of its own ps_x
                    # allocation; DVE assembles both into one SBUF tile
                    bc = ps_x.tile([128, 512], f32, tag="px", name="bc0")
                    nc.tensor.matmul(bc[0:D_HEAD, :], ones_col[:],
                                     box["iz"][0][:], start=True, stop=True)
                    bc_sb = pr.tile([128, 512], f32, tag="bcsb")
                    nc.vector.tensor_copy(bc_sb[0:64, :], bc[0:D_HEAD, :])
                    box["bc"] = bc_sb

                def go_bc1():
                    bc = ps_x.tile([128, 512], f32, tag="px", name="bc1")
                    nc.tensor.matmul(bc[0:D_HEAD, :], ones_col[:],
                                     box["iz"][1][:], start=True, stop=True)
                    nc.vector.tensor_copy(box["bc"][64:128, :], bc[0:D_HEAD, :])

                def go_tt0():
                    nc.vector.tensor_tensor(
                        attnT[0:64, o, sq:sq + 512],
                        av0[0:D_HEAD, :], box["bc"][0:64, :],
                        mybir.AluOpType.mult)

                def go_tt1():
                    nc.vector.tensor_tensor(
                        attnT[64:128, o, sq:sq + 512],
                        av1[0:D_HEAD, :], box["bc"][64:128, :],
                        mybir.AluOpType.mult)

                return [(0, go_recip), (512, go_bc0), (512, go_bc1),
                        (0, go_tt0), (0, go_tt1)]

            cur_av = {}

            def attend_quarter(o, jq, budget, lag=2, step_queue=None):
                h0, h1 = 2 * o, 2 * o + 1
                sq = jq * 512
                av0 = ps_av.tile([128, 512], f32, tag="av", name="av0")
                av1 = ps_av.tile([128, 512], f32, tag="av", name="av1")
                cur_av[(o, jq)] = (av0, av1)

                def emit_av(t, probs):
                    nc.tensor.matmul(
                        av0[0:_VW, :],
                        v_aug[:, t, h0 * _VW:(h0 + 1) * _VW],
                        probs[:, 0:512],
                        start=(t == 0), stop=(t == _NT - 1))
                    nc.tensor.matmul(
                        av1[0:_VW, :],
                        v_aug[:, t, h1 * _VW:(h1 + 1) * _VW],
                        probs[:, 512:1024],
                        start=(t == 0), stop=(t == _NT - 1))

                pend = deque()
                for t in range(_NT):
                    sc = ps_sc.tile([128, 1024], f32, tag="sc", name="sc")
                    nc.tensor.matmul(
                        sc[:, 0:512],
                        kT[0:64, o, t * 128:(t + 1) * 128],
                        qT[0:64, o, sq:sq + 512],
                        start=True, stop=True)
                    nc.tensor.matmul(
                        sc[:, 512:1024],
                        kT[64:128, o, t * 128:(t + 1) * 128],
                        qT[64:128, o, sq:sq + 512],
                        start=True, stop=True)
                    probs = pr.tile([128, 1024], f32r, tag="probs")
                    nc.scalar.activation(probs[:], sc[:], AF.Exp,
                                         scale=float(SCALE))
                    if len(pend) >= lag:
                        emit_av(*pend.popleft())
                    pend.append((t, probs))
                    if step_queue:
                        for go in step_queue.popleft():
                            go()
                        clock[0] += 1.0
                    else:
                        pump(budget)
                while pend:
                    emit_av(*pend.popleft())

            # ---------- output projection ----------
            def out_sg_closures(sg, pools):
                """out rows [sg*128, sg*128+128) = attnT^T @ Wh, 2 K-chunks."""
                box = {}

                def mk(o):
                    def go():
                        if o == 0:
                            box["po"] = pools[sg % len(pools)].tile(
                                [128, 512], f32, tag="px" if pools[sg % len(pools)] is ps_x else "av",
                                name="po")
                        nc.tensor.matmul(
                            box["po"][:],
                            attnT[:, o, sg * 128:(sg + 1) * 128],
                            w_sb["wh"][:, o, :],
                            start=(o == 0), stop=(o == DQ // 128 - 1))
                        if o == DQ // 128 - 1:
                            ot = pr.tile([128, 512], bf16, tag="ot")
                            nc.vector.tensor_copy(ot[:], box["po"][:])
                            nc.sync.dma_start(
                                o_d.rearrange("(t p) d -> p t d", p=128)[:, sg, :],
                                ot[:])
                    return go
                return [(512, mk(o)) for o in range(DQ // 128)]

            # ================= schedule =================
            # PE p-state warmup: tiny K=2 matmuls into the rotator bank keep
            # PE continuously busy from ~0.6us so the 2.4GHz ramp completes
            # before the real projections start.
            if _rep == 0:
                warm = ps_x.tile([128, 512], f32, tag="px", name="warm")
                for _ in range(30):
                    nc.tensor.matmul(warm[0:D_HEAD, 0:D_HEAD], ones_col[:],
                                     ones_col[:], start=True, stop=True)

            # pre-phase: only what exp step 0..3 strictly needs
            proj_group(kT, "wk", "bk", 0, 0, ps_sc)
            proj_group(qT, "wq", "bq", 0, 0, ps_sc)
            project_v(0, ps_sc)
            project_v(1, ps_sc)

            # quarter (0,0) mandatory per-step work: one projection-group
            # closure per step (kT sg1..3, then qT o0 sg1) + one v tile
            jq0_proj = deque()
            for sg in (1, 2, 3):
                jq0_proj.extend(go for _, go in
                                proj_group_closures(kT, "wk", "bk", 0, sg))
            jq0_proj.extend(go for _, go in
                            proj_group_closures(qT, "wq", "bq", 0, 1))
            step_queue = deque()
            for t in range(_NT):
                items = []
                if t + 2 < _NT:
                    items.append(lambda t=t: project_v(t + 2, ps_sc))
                if jq0_proj:
                    items.append(jq0_proj.popleft())
                step_queue.append(items)

            # fillers for the rest of pair 0: qT o0 sg2..3, kT o1, qT o1
            for sg in (2, 3):
                fillers.extend(proj_group_closures(qT, "wq", "bq", 0, sg))
            for sg in range(4):
                fillers.extend(proj_group_closures(kT, "wk", "bk", 1, sg))
            for sg in range(4):
                fillers.extend(proj_group_closures(qT, "wq", "bq", 1, sg))

            BUDGET = 430  # PE filler rows per exp-step of slack
            attend_quarter(0, 0, BUDGET, step_queue=step_queue)
            for item in reversed(normalize_closures(0, 0)):
                fillers.appendleft(item)
            for jq in range(1, 4):
                attend_quarter(0, jq, BUDGET)
                for item in reversed(normalize_closures(0, jq)):
                    fillers.appendleft(item)
            for jq in range(4):
                if jq > 0:
                    for sg in range(4 * (jq - 1), 4 * jq):
                        fillers.extend(out_sg_closures(sg, [ps_x]))
                attend_quarter(1, jq, BUDGET, lag=2 if jq < 3 else 1)
                for item in reversed(normalize_closures(1, jq)):
                    fillers.appendleft(item)
            # tail: remaining fillers, then the final output quarter using
            # both the rotator and the (now idle) av pool banks
            drain_fillers()
            for sg in range(12, 16):
                for rows, go in out_sg_closures(sg, [ps_av]):
                    go()

    _split_excess_waits(nc)
    return nc


def _in_maps(inputs):
    import ml_dtypes
    x = np.ascontiguousarray(np.asarray(inputs["x"], dtype=np.float32))
    maps = []
    for c in range(N_CORES):
        b, g = c // 2, c % 2
        hs = slice(g * DQ, (g + 1) * DQ)
        maps.append({
            "x": np.ascontiguousarray(x[b].T.astype(ml_dtypes.bfloat16)),
            "wq": np.ascontiguousarray(np.asarray(inputs["Wq"], np.float32)[:, hs].astype(ml_dtypes.bfloat16)),
            "wk": np.ascontiguousarray(np.asarray(inputs["Wk"], np.float32)[:, hs].astype(ml_dtypes.bfloat16)),
            "wv": np.ascontiguousarray(np.asarray(inputs["Wv"], np.float32)[:, hs].astype(ml_dtypes.bfloat16)),
            "wh": np.ascontiguousarray(np.asarray(inputs["Wh"], np.float32)[hs, :].astype(ml_dtypes.bfloat16)),
            "bq": np.ascontiguousarray(np.asarray(inputs["bq"], np.float32)[hs]),
            "bk": np.ascontiguousarray(np.asarray(inputs["bk"], np.float32)[hs]),
            "bv": np.ascontiguousarray(np.asarray(inputs["bv"], np.float32)[hs]),
        })
    return maps


def kernel(**inputs):
    from concourse.bass_utils import run_bass_kernel_spmd

    nc = build_nc(nrep=1)
    maps = _in_maps(inputs)
    res = run_bass_kernel_spmd(nc, maps, core_ids=list(range(N_CORES)))
    bh = (np.asarray(inputs["bh"], np.float32)
          + np.asarray(inputs["bv"], np.float32)
          @ np.asarray(inputs["Wh"], np.float32))
    out = np.empty((B, S, D_MODEL), np.float32)
    for b in range(B):
        out[b] = (res.results[2 * b]["o"].astype(np.float32)
                  + res.results[2 * b + 1]["o"].astype(np.float32) + bh)
    return out
